# revision 7
# baseline (speedup 1.0000x reference)
"""Cross-modal triplet loss (margin ranking on hardest pos/neg pairs) on 8 trn2 NeuronCores.

Strategy (per sharding hint): shard rows of modal1 across the 8 cores (512 rows
each); replicate modal2 and targets. Each core computes its 512x4096 slab of

    psum[m, j] = dot(m1[m], m2[j]) - sq2[j]/2 - (BIG/2) * mask[m, j]

entirely on the PE array: the main dot-product runs as fp8(e4m3) matmuls in
DoubleRow perf mode (two 128-row k-tiles per instruction), and the
sq2/same-identity-mask terms ride along as one small bf16 "aug" matmul per
accumulation group (the mask is rank-64 over the 64 ids: -8192*onehot1 x
onehot2; -sq2/2 enters as a bf16 hi/lo pair for ~16-bit accuracy).

Row-wise min(psum) then locates the hardest positive (same-id entries sit
-8192 below all diff-id entries) and max(psum) the hardest negative:

    ap^2 = sq1[m] - 2*min_j(psum) - BIG      (hardest-positive distance^2)
    an^2 = sq1[m] - 2*max_j(psum)            (hardest-negative distance^2)

All operand layout work (k-major transposes, fp8/bf16 casts, one-hot mask
construction, sq1/sq2 row norms) happens on the host while preparing the shard
buffers, so the device program is nothing but DMA-in -> matmul -> row-reduce ->
DMA-out.  The output columns are processed in passes of 1 or 2 chunks
(4/8 PSUM banks); modal2 arrives as one contiguous pass-major block per pass,
so the first matmul only waits for a 1MB DMA and DMA stays ahead of the PE
thereafter.  Each weight tile is streamed against all of the pass's PSUM banks
so LDWEIGHTS amortizes away.  Scalar-engine copies evacuate finished PSUM
banks (freeing them for the next pass) and the DVE reduces min/max from the
SBUF copies off the critical path; the narrow last pass reduces straight from
PSUM to keep the tail short.  The per-row psum min/max (512 rows x 2 values
per core) return to the host, which applies the sq1 shift, sqrt, margin hinge,
and the mean over all 4096 rows.
"""

import functools

import ml_dtypes
import numpy as np

import concourse.bass as bass
import concourse.mybir as mybir
import concourse.tile as tile
from concourse import bacc
from concourse.bass_utils import run_bass_kernel_spmd

F32 = mybir.dt.float32
BF16 = mybir.dt.bfloat16
F8 = mybir.dt.float8e4
OP = mybir.AluOpType
AX = mybir.AxisListType.X
DR = mybir.MatmulPerfMode.DoubleRow

NP_F8 = ml_dtypes.float8_e4m3
NP_BF16 = ml_dtypes.bfloat16

N, D, NIDS, P = 4096, 2048, 64, 128
NCORES = 8
SH = N // NCORES      # 512 rows of modal1 per core
MT = SH // P          # 4 m-tiles per core
KT = D // P           # 16 k-tiles
KT2 = KT // 2         # 8 double-k-tiles (DoubleRow covers 256 of K each)
CHUNK = 512           # output columns per PSUM bank
NJC = N // CHUNK      # 8 column chunks
KAUG = 66             # 64 one-hot mask rows + sq2 hi/lo
BIG = 16384.0         # separates same-id from diff-id psum values
EPS = 1e-12

PASS_W = [1, 2, 2, 2, 1]          # chunks per pass (4*W PSUM banks each)
PASS_O = [0, 1, 3, 5, 7]          # chunk offset of each pass


def _build() -> bass.Bass:
    nc = bacc.Bacc(num_swdge_queues=4)
    m1d = nc.dram_tensor("m1dr", [P, KT, SH], F8, kind="ExternalInput")
    m2ds = [
        nc.dram_tensor(f"m2p{pc}", [P, KT, w * CHUNK], F8, kind="ExternalInput")
        for pc, w in enumerate(PASS_W)
    ]
    laugd = nc.dram_tensor("laug", [KAUG, SH], BF16, kind="ExternalInput")
    raugd = nc.dram_tensor("raug", [KAUG, N], BF16, kind="ExternalInput")
    outd = nc.dram_tensor("out", [P, 2 * MT], F32, kind="ExternalOutput")

    with tile.TileContext(nc) as tc:
        with (
            tc.tile_pool(name="const", bufs=1) as const,
            tc.tile_pool(name="ev", bufs=8) as evp,
            tc.tile_pool(name="ps", bufs=8, space=bass.MemorySpace.PSUM) as psp,
        ):
            m1sb = const.tile([P, KT, SH], F8)
            nc.sync.dma_start(m1sb[:, :, :], m1d[:, :, :])
            laug = const.tile([KAUG, SH], BF16)
            nc.gpsimd.dma_start(laug[:, :], laugd[:, :])
            raug = const.tile([KAUG, N], BF16)
            nc.gpsimd.dma_start(raug[:, :], raugd[:, :])

            # one contiguous pass-major DMA per pass; sync/gpsimd alternate so
            # both trigger paths stay busy and pass pc+1 lands before the PE
            # finishes pass pc.
            m2sb = []
            for pc, w in enumerate(PASS_W):
                m2p = const.tile([P, KT, w * CHUNK], F8, name=f"m2sb{pc}")
                eng = nc.sync if pc % 2 == 0 else nc.gpsimd
                eng.dma_start(m2p[:, :, :], m2ds[pc][:, :, :])
                m2sb.append(m2p)

            minb = [
                const.tile([P, NJC], F32, name=f"minb{mt}") for mt in range(MT)
            ]
            maxb = [
                const.tile([P, NJC], F32, name=f"maxb{mt}") for mt in range(MT)
            ]

            last = len(PASS_W) - 1
            for pc, w in enumerate(PASS_W):
                banks = []  # (mt, jc, psum tile)
                for mt in range(MT):
                    for j in range(w):
                        jc = PASS_O[pc] + j
                        banks.append(
                            (
                                mt,
                                jc,
                                psp.tile(
                                    [P, CHUNK], F32, tag="ps", name=f"ps{pc}_{mt}_{jc}"
                                ),
                            )
                        )
                for t in range(KT2):
                    for mt, jc, ps in banks:
                        j = jc - PASS_O[pc]
                        nc.tensor.matmul(
                            ps[:, :],
                            m1sb[:, 2 * t : 2 * t + 2, mt * P : (mt + 1) * P],
                            m2sb[pc][:, 2 * t : 2 * t + 2, j * CHUNK : (j + 1) * CHUNK],
                            start=(t == 0),
                            stop=False,
                            perf_mode=DR,
                        )
                for mt, jc, ps in banks:
                    nc.tensor.matmul(
                        ps[:, :],
                        laug[:, mt * P : (mt + 1) * P],
                        raug[:, jc * CHUNK : (jc + 1) * CHUNK],
                        start=False,
                        stop=True,
                    )
                if pc != last:
                    # Scalar-engine copy evacuates each PSUM bank (frees it
                    # for the next pass); DVE reduces from the SBUF copy.
                    evs = []
                    for mt, jc, ps in banks:
                        ev = evp.tile(
                            [P, CHUNK], F32, tag="ev", name=f"ev{pc}_{mt}_{jc}"
                        )
                        nc.scalar.copy(ev[:, :], ps[:, :])
                        evs.append((mt, jc, ev))
                    for mt, jc, ev in evs:
                        nc.vector.tensor_reduce(
                            minb[mt][:, jc : jc + 1], ev[:, :], AX, OP.min
                        )
                        nc.vector.tensor_reduce(
                            maxb[mt][:, jc : jc + 1], ev[:, :], AX, OP.max
                        )
                else:
                    # final pass: no one needs the banks back; reduce directly
                    for mt, jc, ps in banks:
                        nc.vector.tensor_reduce(
                            minb[mt][:, jc : jc + 1], ps[:, :], AX, OP.min
                        )
                        nc.vector.tensor_reduce(
                            maxb[mt][:, jc : jc + 1], ps[:, :], AX, OP.max
                        )

            osb = const.tile([P, 2 * MT], F32)
            for mt in range(MT):
                nc.vector.tensor_reduce(
                    osb[:, mt : mt + 1], minb[mt][:, :], AX, OP.min
                )
                nc.vector.tensor_reduce(
                    osb[:, MT + mt : MT + mt + 1], maxb[mt][:, :], AX, OP.max
                )
            nc.sync.dma_start(outd[:, :], osb[:, :])

    nc.finalize()
    return nc


@functools.lru_cache(maxsize=1)
def _get_program() -> bass.Bass:
    return _build()


def _make_in_maps(m1, m2, targets):
    ids = np.arange(NIDS)
    tgt = np.asarray(targets).astype(np.int64).reshape(N)

    # k-major fp8 operand layout: tile[p, s, x] = X[x, s*128 + p],
    # then split column-wise into contiguous pass-major blocks
    m2dr = np.ascontiguousarray(
        m2.astype(NP_F8).reshape(N, KT, P).transpose(2, 1, 0)
    )
    m2blocks = {
        f"m2p{pc}": np.ascontiguousarray(
            m2dr[:, :, PASS_O[pc] * CHUNK : (PASS_O[pc] + w) * CHUNK]
        )
        for pc, w in enumerate(PASS_W)
    }

    # right aug features (shared): one-hot ids + sq2 hi/lo (bf16 pair)
    sq2h = -0.5 * np.sum(m2.astype(np.float64) * m2.astype(np.float64), axis=1)
    shi = sq2h.astype(np.float32).astype(NP_BF16)
    slo = (sq2h - shi.astype(np.float64)).astype(np.float32).astype(NP_BF16)
    raug = np.zeros((KAUG, N), dtype=NP_BF16)
    raug[:NIDS] = (tgt[None, :] == ids[:, None]).astype(NP_BF16)
    raug[NIDS] = shi
    raug[NIDS + 1] = slo
    raug = np.ascontiguousarray(raug)

    maps = []
    for c in range(NCORES):
        m1c = m1[c * SH : (c + 1) * SH]
        m1dr = np.ascontiguousarray(
            m1c.astype(NP_F8).reshape(SH, KT, P).transpose(2, 1, 0)
        )
        tgtc = tgt[c * SH : (c + 1) * SH]
        laug = np.zeros((KAUG, SH), dtype=NP_BF16)
        laug[:NIDS] = (-BIG / 2.0) * (tgtc[None, :] == ids[:, None]).astype(
            np.float32
        )
        laug[NIDS] = 1.0
        laug[NIDS + 1] = 1.0
        maps.append(
            {
                "m1dr": m1dr,
                "laug": np.ascontiguousarray(laug),
                "raug": raug,
                **m2blocks,
            }
        )
    return maps


def run(modal1_inputs, modal2_inputs, targets, margin, trace=False):
    m1 = np.ascontiguousarray(np.asarray(modal1_inputs, dtype=np.float32))
    m2 = np.ascontiguousarray(np.asarray(modal2_inputs, dtype=np.float32))
    nc = _get_program()
    res = run_bass_kernel_spmd(
        nc, _make_in_maps(m1, m2, targets), list(range(NCORES)), trace=trace
    )

    # host finale: undo the psum encoding, sqrt, hinge, mean
    pmin = np.empty(N, dtype=np.float64)
    pmax = np.empty(N, dtype=np.float64)
    for c, r in enumerate(res.results):
        o = np.asarray(r["out"], dtype=np.float64)  # [P, 2*MT]
        pmin[c * SH : (c + 1) * SH] = o[:, :MT].T.reshape(SH)
        pmax[c * SH : (c + 1) * SH] = o[:, MT:].T.reshape(SH)

    sq1 = np.sum(m1.astype(np.float64) * m1.astype(np.float64), axis=1)
    ap2 = np.maximum(sq1 - 2.0 * pmin - BIG, EPS)
    an2 = np.maximum(sq1 - 2.0 * pmax, EPS)
    ap = np.sqrt(ap2)
    an = np.sqrt(an2)
    loss = np.float32(np.mean(np.maximum(ap - an + float(margin), 0.0)))
    prec = np.float32(np.mean(an > ap))
    return (loss, prec), res


def kernel(modal1_inputs, modal2_inputs, targets, margin):
    (loss, prec), _ = run(modal1_inputs, modal2_inputs, targets, margin)
    return loss, prec


# revision 9
# speedup vs baseline: 1.1067x; 1.1067x over previous
"""Cross-modal triplet loss (margin ranking on hardest pos/neg pairs) on 8 trn2 NeuronCores.

Strategy (per sharding hint): shard rows of modal1 across the 8 cores (512 rows
each); replicate modal2 and targets. Each core computes its 512x4096 slab of

    psum[m, j] = dot(m1[m], m2[j]) - sq2[j]/2 - (BIG/2) * mask[m, j]

entirely on the PE array: the main dot-product runs as fp8(e4m3) matmuls in
DoubleRow perf mode (two 128-row k-tiles per instruction), and the
sq2/same-identity-mask terms ride along as one small bf16 "aug" matmul per
accumulation group (the mask is rank-64 over the 64 ids: -8192*onehot1 x
onehot2; -sq2/2 enters as a bf16 hi/lo pair for ~16-bit accuracy).

Row-wise min(psum) then locates the hardest positive (same-id entries sit
-8192 below all diff-id entries) and max(psum) the hardest negative:

    ap^2 = sq1[m] - 2*min_j(psum) - BIG      (hardest-positive distance^2)
    an^2 = sq1[m] - 2*max_j(psum)            (hardest-negative distance^2)

All operand layout work (k-major transposes, fp8/bf16 casts, one-hot mask
construction, sq1/sq2 row norms) happens on the host while preparing the shard
buffers, so the device program is nothing but DMA-in -> matmul -> row-reduce ->
DMA-out.  The output columns are processed in passes of 1 or 2 chunks
(4/8 PSUM banks); modal2 arrives as one contiguous pass-major block per pass,
so the first matmul only waits for a 1MB DMA and DMA stays ahead of the PE
thereafter.  Each weight tile is streamed against all of the pass's PSUM banks
so LDWEIGHTS amortizes away.  Scalar-engine copies evacuate finished PSUM
banks (freeing them for the next pass) and the DVE reduces min/max from the
SBUF copies off the critical path; the narrow last pass reduces straight from
PSUM to keep the tail short.  The per-row psum min/max (512 rows x 2 values
per core) return to the host, which applies the sq1 shift, sqrt, margin hinge,
and the mean over all 4096 rows.
"""

import functools

import ml_dtypes
import numpy as np

import concourse.bass as bass
import concourse.mybir as mybir
import concourse.tile as tile
from concourse import bacc
from concourse.bass_utils import run_bass_kernel_spmd

F32 = mybir.dt.float32
BF16 = mybir.dt.bfloat16
F8 = mybir.dt.float8e4
OP = mybir.AluOpType
AX = mybir.AxisListType.X
DR = mybir.MatmulPerfMode.DoubleRow

NP_F8 = ml_dtypes.float8_e4m3
NP_BF16 = ml_dtypes.bfloat16

N, D, NIDS, P = 4096, 2048, 64, 128
NCORES = 8
SH = N // NCORES      # 512 rows of modal1 per core
MT = SH // P          # 4 m-tiles per core
KT = D // P           # 16 k-tiles
KT2 = KT // 2         # 8 double-k-tiles (DoubleRow covers 256 of K each)
CHUNK = 512           # output columns per PSUM bank
NJC = N // CHUNK      # 8 column chunks
KAUG = 66             # 64 one-hot mask rows + sq2 hi/lo
BIG = 16384.0         # separates same-id from diff-id psum values
EPS = 1e-12

PASS_W = [1, 2, 2, 2, 1]          # chunks per pass (4*W PSUM banks each)
PASS_O = [0, 1, 3, 5, 7]          # chunk offset of each pass


def _build() -> bass.Bass:
    nc = bacc.Bacc(num_swdge_queues=4)
    m1d = nc.dram_tensor("m1dr", [P, KT, SH], F8, kind="ExternalInput")
    m2ds = [
        nc.dram_tensor(f"m2p{pc}", [P, KT, w * CHUNK], F8, kind="ExternalInput")
        for pc, w in enumerate(PASS_W)
    ]
    laugd = nc.dram_tensor("laug", [KAUG, SH], BF16, kind="ExternalInput")
    raugd = nc.dram_tensor("raug", [KAUG, N], BF16, kind="ExternalInput")
    outd = nc.dram_tensor("out", [P, 2 * MT], F32, kind="ExternalOutput")

    with tile.TileContext(nc) as tc:
        with (
            tc.tile_pool(name="const", bufs=1) as const,
            tc.tile_pool(name="ev", bufs=8) as evp,
            tc.tile_pool(name="ps", bufs=8, space=bass.MemorySpace.PSUM) as psp,
        ):
            # All bulk traffic rides the two hardware DGE queues (sync +
            # scalar), split into k-halves and issued in consumption order;
            # the software DGE (gpsimd) only carries the small aug tensors.
            KH = KT // 2
            m1sb = const.tile([P, KT, SH], F8)
            nc.sync.dma_start(m1sb[:, 0:KH, :], m1d[:, 0:KH, :])
            nc.scalar.dma_start(m1sb[:, KH:KT, :], m1d[:, KH:KT, :])
            laug = const.tile([KAUG, SH], BF16)
            nc.gpsimd.dma_start(laug[:, :], laugd[:, :])
            raug = const.tile([KAUG, N], BF16)
            nc.gpsimd.dma_start(raug[:, :], raugd[:, :])

            m2sb = []
            for pc, w in enumerate(PASS_W):
                m2p = const.tile([P, KT, w * CHUNK], F8, name=f"m2sb{pc}")
                nc.sync.dma_start(m2p[:, 0:KH, :], m2ds[pc][:, 0:KH, :])
                nc.scalar.dma_start(m2p[:, KH:KT, :], m2ds[pc][:, KH:KT, :])
                m2sb.append(m2p)

            minb = [
                const.tile([P, NJC], F32, name=f"minb{mt}") for mt in range(MT)
            ]
            maxb = [
                const.tile([P, NJC], F32, name=f"maxb{mt}") for mt in range(MT)
            ]

            def mm(pc, t, mt, jc, ps):
                j = jc - PASS_O[pc]
                nc.tensor.matmul(
                    ps[:, :],
                    m1sb[:, 2 * t : 2 * t + 2, mt * P : (mt + 1) * P],
                    m2sb[pc][:, 2 * t : 2 * t + 2, j * CHUNK : (j + 1) * CHUNK],
                    start=(t == 0),
                    stop=False,
                    perf_mode=DR,
                )

            def mm_aug(mt, jc, ps):
                nc.tensor.matmul(
                    ps[:, :],
                    laug[:, mt * P : (mt + 1) * P],
                    raug[:, jc * CHUNK : (jc + 1) * CHUNK],
                    start=False,
                    stop=True,
                )

            def evac(pc, mt, jc, ps, direct):
                # Scalar-engine copy evacuates the PSUM bank (freeing it for
                # the next allocation); DVE reduces from the SBUF copy, off
                # the bank critical path. The final pass reduces straight from
                # PSUM: nobody needs those banks back.
                src = ps
                if not direct:
                    ev = evp.tile(
                        [P, CHUNK], F32, tag="ev", name=f"ev{pc}_{mt}_{jc}"
                    )
                    nc.scalar.copy(ev[:, :], ps[:, :])
                    src = ev
                nc.vector.tensor_reduce(
                    minb[mt][:, jc : jc + 1], src[:, :], AX, OP.min
                )
                nc.vector.tensor_reduce(
                    maxb[mt][:, jc : jc + 1], src[:, :], AX, OP.max
                )

            last = len(PASS_W) - 1
            for pc, w in enumerate(PASS_W):
                banks = [
                    (
                        mt,
                        PASS_O[pc] + j,
                        psp.tile(
                            [P, CHUNK], F32, tag="ps", name=f"ps{pc}_{mt}_{j}"
                        ),
                    )
                    for mt in range(MT)
                    for j in range(w)
                ]
                if pc == 0:
                    # t-major: the first matmul only needs the first k-slice
                    # of the pass-0 block, so the PE starts ~1MB earlier.
                    for t in range(KT2):
                        for mt, jc, ps in banks:
                            mm(pc, t, mt, jc, ps)
                    for mt, jc, ps in banks:
                        mm_aug(mt, jc, ps)
                    for mt, jc, ps in banks:
                        evac(pc, mt, jc, ps, direct=False)
                else:
                    # bank-major: each bank runs its full accumulation chain
                    # and is evacuated immediately, so PSUM banks recycle
                    # continuously and pass boundaries never stall.
                    for mt, jc, ps in banks:
                        for t in range(KT2):
                            mm(pc, t, mt, jc, ps)
                        mm_aug(mt, jc, ps)
                        evac(pc, mt, jc, ps, direct=(pc == last))

            osb = const.tile([P, 2 * MT], F32)
            for mt in range(MT):
                nc.vector.tensor_reduce(
                    osb[:, mt : mt + 1], minb[mt][:, :], AX, OP.min
                )
                nc.vector.tensor_reduce(
                    osb[:, MT + mt : MT + mt + 1], maxb[mt][:, :], AX, OP.max
                )
            nc.sync.dma_start(outd[:, :], osb[:, :])

    nc.finalize()
    return nc


@functools.lru_cache(maxsize=1)
def _get_program() -> bass.Bass:
    return _build()


def _make_in_maps(m1, m2, targets):
    ids = np.arange(NIDS)
    tgt = np.asarray(targets).astype(np.int64).reshape(N)

    # k-major fp8 operand layout: tile[p, s, x] = X[x, s*128 + p],
    # then split column-wise into contiguous pass-major blocks
    m2dr = np.ascontiguousarray(
        m2.astype(NP_F8).reshape(N, KT, P).transpose(2, 1, 0)
    )
    m2blocks = {
        f"m2p{pc}": np.ascontiguousarray(
            m2dr[:, :, PASS_O[pc] * CHUNK : (PASS_O[pc] + w) * CHUNK]
        )
        for pc, w in enumerate(PASS_W)
    }

    # right aug features (shared): one-hot ids + sq2 hi/lo (bf16 pair)
    sq2h = -0.5 * np.sum(m2.astype(np.float64) * m2.astype(np.float64), axis=1)
    shi = sq2h.astype(np.float32).astype(NP_BF16)
    slo = (sq2h - shi.astype(np.float64)).astype(np.float32).astype(NP_BF16)
    raug = np.zeros((KAUG, N), dtype=NP_BF16)
    raug[:NIDS] = (tgt[None, :] == ids[:, None]).astype(NP_BF16)
    raug[NIDS] = shi
    raug[NIDS + 1] = slo
    raug = np.ascontiguousarray(raug)

    maps = []
    for c in range(NCORES):
        m1c = m1[c * SH : (c + 1) * SH]
        m1dr = np.ascontiguousarray(
            m1c.astype(NP_F8).reshape(SH, KT, P).transpose(2, 1, 0)
        )
        tgtc = tgt[c * SH : (c + 1) * SH]
        laug = np.zeros((KAUG, SH), dtype=NP_BF16)
        laug[:NIDS] = (-BIG / 2.0) * (tgtc[None, :] == ids[:, None]).astype(
            np.float32
        )
        laug[NIDS] = 1.0
        laug[NIDS + 1] = 1.0
        maps.append(
            {
                "m1dr": m1dr,
                "laug": np.ascontiguousarray(laug),
                "raug": raug,
                **m2blocks,
            }
        )
    return maps


def run(modal1_inputs, modal2_inputs, targets, margin, trace=False):
    m1 = np.ascontiguousarray(np.asarray(modal1_inputs, dtype=np.float32))
    m2 = np.ascontiguousarray(np.asarray(modal2_inputs, dtype=np.float32))
    nc = _get_program()
    res = run_bass_kernel_spmd(
        nc, _make_in_maps(m1, m2, targets), list(range(NCORES)), trace=trace
    )

    # host finale: undo the psum encoding, sqrt, hinge, mean
    pmin = np.empty(N, dtype=np.float64)
    pmax = np.empty(N, dtype=np.float64)
    for c, r in enumerate(res.results):
        o = np.asarray(r["out"], dtype=np.float64)  # [P, 2*MT]
        pmin[c * SH : (c + 1) * SH] = o[:, :MT].T.reshape(SH)
        pmax[c * SH : (c + 1) * SH] = o[:, MT:].T.reshape(SH)

    sq1 = np.sum(m1.astype(np.float64) * m1.astype(np.float64), axis=1)
    ap2 = np.maximum(sq1 - 2.0 * pmin - BIG, EPS)
    an2 = np.maximum(sq1 - 2.0 * pmax, EPS)
    ap = np.sqrt(ap2)
    an = np.sqrt(an2)
    loss = np.float32(np.mean(np.maximum(ap - an + float(margin), 0.0)))
    prec = np.float32(np.mean(an > ap))
    return (loss, prec), res


def kernel(modal1_inputs, modal2_inputs, targets, margin):
    (loss, prec), _ = run(modal1_inputs, modal2_inputs, targets, margin)
    return loss, prec


# revision 14
# speedup vs baseline: 1.1298x; 1.0209x over previous
"""Cross-modal triplet loss (margin ranking on hardest pos/neg pairs) on 8 trn2 NeuronCores.

Strategy (per sharding hint): shard rows of modal1 across the 8 cores (512 rows
each); replicate modal2 and targets. Each core computes its 512x4096 slab of

    psum[m, j] = dot(m1[m], m2[j]) - sq2[j]/2 - (BIG/2) * mask[m, j]

entirely on the PE array: the main dot-product runs as fp8(e4m3) matmuls in
DoubleRow perf mode (two 128-row k-tiles per instruction), and the
sq2/same-identity-mask terms ride along as one small bf16 "aug" matmul per
accumulation group (the mask is rank-64 over the 64 ids: -8192*onehot1 x
onehot2; -sq2/2 enters as a bf16 hi/lo pair for ~16-bit accuracy).

Row-wise min(psum) then locates the hardest positive (same-id entries sit
-8192 below all diff-id entries) and max(psum) the hardest negative:

    ap^2 = sq1[m] - 2*min_j(psum) - BIG      (hardest-positive distance^2)
    an^2 = sq1[m] - 2*max_j(psum)            (hardest-negative distance^2)

All operand layout work (k-major transposes, fp8/bf16 casts, one-hot mask
construction, sq1/sq2 row norms) happens on the host while preparing the shard
buffers, so the device program is nothing but DMA-in -> matmul -> row-reduce ->
DMA-out.  The output columns are processed in passes of 1 or 2 chunks
(4/8 PSUM banks); modal2 arrives as one contiguous pass-major block per pass,
so the first matmul only waits for a 1MB DMA and DMA stays ahead of the PE
thereafter.  Each weight tile is streamed against all of the pass's PSUM banks
so LDWEIGHTS amortizes away.  Scalar-engine copies evacuate finished PSUM
banks (freeing them for the next pass) and the DVE reduces min/max from the
SBUF copies off the critical path; the narrow last pass reduces straight from
PSUM to keep the tail short.  The per-row psum min/max (512 rows x 2 values
per core) return to the host, which applies the sq1 shift, sqrt, margin hinge,
and the mean over all 4096 rows.
"""

import functools

import ml_dtypes
import numpy as np

import concourse.bass as bass
import concourse.mybir as mybir
import concourse.tile as tile
from concourse import bacc
from concourse.bass_utils import run_bass_kernel_spmd

F32 = mybir.dt.float32
BF16 = mybir.dt.bfloat16
F8 = mybir.dt.float8e4
OP = mybir.AluOpType
AX = mybir.AxisListType.X
DR = mybir.MatmulPerfMode.DoubleRow

NP_F8 = ml_dtypes.float8_e4m3
NP_BF16 = ml_dtypes.bfloat16

N, D, NIDS, P = 4096, 2048, 64, 128
NCORES = 8
SH = N // NCORES      # 512 rows of modal1 per core
MT = SH // P          # 4 m-tiles per core
KT = D // P           # 16 k-tiles
KT2 = KT // 2         # 8 double-k-tiles (DoubleRow covers 256 of K each)
CHUNK = 512           # output columns per PSUM bank
NJC = N // CHUNK      # 8 column chunks
KAUG = 66             # 64 one-hot mask rows + sq2 hi/lo
BIG = 16384.0         # separates same-id from diff-id psum values
EPS = 1e-12

PASS_W = [1, 2, 2, 2, 1]          # chunks per pass (4*W PSUM banks each)
PASS_O = [0, 1, 3, 5, 7]          # chunk offset of each pass


def _build() -> bass.Bass:
    nc = bacc.Bacc(num_swdge_queues=4)
    m1d = nc.dram_tensor("m1dr", [P, KT, SH], F8, kind="ExternalInput")
    m2ds = [
        nc.dram_tensor(f"m2p{pc}", [P, KT, w * CHUNK], F8, kind="ExternalInput")
        for pc, w in enumerate(PASS_W)
    ]
    laugd = nc.dram_tensor("laug", [KAUG, SH], BF16, kind="ExternalInput")
    raugd = nc.dram_tensor("raug", [KAUG, N], BF16, kind="ExternalInput")
    outd = nc.dram_tensor("out", [P, 2 * MT], F32, kind="ExternalOutput")

    with tile.TileContext(nc) as tc:
        with (
            tc.tile_pool(name="const", bufs=1) as const,
            tc.tile_pool(name="ev", bufs=1) as evp,
            tc.tile_pool(name="ps", bufs=1, space=bass.MemorySpace.PSUM) as psp,
        ):
            # DMA choreography. Each trigger queue (sync-HWDGE, scalar-HWDGE,
            # gpsimd's 4 software queues) only sustains ~100GB/s and starts at
            # a staggered time, so pieces are sized/placed by deadline: the
            # first k-pair slices of m1+pass0 ride sync (earliest queue), the
            # next ones scalar, the late pass-0 slices + all bulk pass blocks
            # fan out over the software queues, everything in consumption
            # order.
            KH = KT // 2
            m1sb = const.tile([P, KT, SH], F8)
            m2sb = [
                const.tile([P, KT, w * CHUNK], F8, name=f"m2sb{pc}")
                for pc, w in enumerate(PASS_W)
            ]
            laug = const.tile([KAUG, SH], BF16)
            raug = const.tile([KAUG, N], BF16)

            def tpair(eng, t):
                eng.dma_start(
                    m1sb[:, 2 * t : 2 * t + 2, :], m1d[:, 2 * t : 2 * t + 2, :]
                )
                eng.dma_start(
                    m2sb[0][:, 2 * t : 2 * t + 2, :], m2ds[0][:, 2 * t : 2 * t + 2, :]
                )

            # sync: pass-0 head + all aug operands
            tpair(nc.sync, 0)
            tpair(nc.sync, 1)
            nc.sync.dma_start(laug[:, :], laugd[:, :])
            nc.sync.dma_start(raug[:, 0:CHUNK], raugd[:, 0:CHUNK])
            nc.sync.dma_start(raug[:, 3 * CHUNK : N], raugd[:, 3 * CHUNK : N])
            # scalar: pass-0 middle, then mid-deadline bulk
            tpair(nc.scalar, 2)
            tpair(nc.scalar, 3)
            nc.scalar.dma_start(
                raug[:, CHUNK : 3 * CHUNK], raugd[:, CHUNK : 3 * CHUNK]
            )
            nc.scalar.dma_start(m2sb[2][:, 0:KH, :], m2ds[2][:, 0:KH, :])
            nc.scalar.dma_start(m2sb[4][:, 0:KH, :], m2ds[4][:, 0:KH, :])
            # software queues: pass-0 tail in parallel, then the bulk blocks
            for t in range(4, KT2):
                tpair(nc.gpsimd, t)
            for q in range(4):
                nc.gpsimd.dma_start(
                    m2sb[1][:, 4 * q : 4 * q + 4, :], m2ds[1][:, 4 * q : 4 * q + 4, :]
                )
            nc.gpsimd.dma_start(m2sb[2][:, KH:KT, :], m2ds[2][:, KH:KT, :])
            nc.gpsimd.dma_start(m2sb[3][:, 0:KH, :], m2ds[3][:, 0:KH, :])
            nc.gpsimd.dma_start(m2sb[3][:, KH:KT, :], m2ds[3][:, KH:KT, :])
            nc.gpsimd.dma_start(m2sb[4][:, KH:KT, :], m2ds[4][:, KH:KT, :])

            minb = [
                const.tile([P, NJC], F32, name=f"minb{mt}") for mt in range(MT)
            ]
            maxb = [
                const.tile([P, NJC], F32, name=f"maxb{mt}") for mt in range(MT)
            ]

            def mm(pc, t, mt, jc, ps):
                j = jc - PASS_O[pc]
                nc.tensor.matmul(
                    ps[:, :],
                    m1sb[:, 2 * t : 2 * t + 2, mt * P : (mt + 1) * P],
                    m2sb[pc][:, 2 * t : 2 * t + 2, j * CHUNK : (j + 1) * CHUNK],
                    start=(t == 0),
                    stop=False,
                    perf_mode=DR,
                )

            def mm_aug(mt, jc, ps):
                nc.tensor.matmul(
                    ps[:, :],
                    laug[:, mt * P : (mt + 1) * P],
                    raug[:, jc * CHUNK : (jc + 1) * CHUNK],
                    start=False,
                    stop=True,
                )

            # 8 PSUM banks + 8 eviction buffers, allocated once and rotated
            # manually (the framework's per-allocation bookkeeping semaphores
            # otherwise drain serially at context exit). WAR hazards on reuse
            # are still tracked: a start=True matmul waits for the previous
            # occupant's eviction read.
            pss = [psp.tile([P, CHUNK], F32, name=f"psb{i}") for i in range(8)]
            evt = [evp.tile([P, CHUNK], F32, name=f"evb{i}") for i in range(8)]

            def evac(mt, jc, ps, ev, direct):
                # Scalar-engine copy evacuates the PSUM bank (freeing it for
                # the next pass); DVE reduces from the SBUF copy, off the
                # bank critical path. The final pass reduces straight from
                # PSUM: nobody needs those banks back.
                src = ps
                if not direct:
                    nc.scalar.copy(ev[:, :], ps[:, :])
                    src = ev
                nc.vector.tensor_reduce(
                    minb[mt][:, jc : jc + 1], src[:, :], AX, OP.min
                )
                nc.vector.tensor_reduce(
                    maxb[mt][:, jc : jc + 1], src[:, :], AX, OP.max
                )

            last = len(PASS_W) - 1
            ctr = 0
            for pc, w in enumerate(PASS_W):
                banks = []
                for mt in range(MT):
                    for j in range(w):
                        banks.append(
                            (mt, PASS_O[pc] + j, pss[ctr % 8], evt[ctr % 8])
                        )
                        ctr += 1
                if pc == 0:
                    # t-major: the first matmul only needs the first k-slice
                    # of the pass-0 block, so the PE starts ~1MB earlier.
                    for t in range(KT2):
                        for mt, jc, ps, ev in banks:
                            mm(pc, t, mt, jc, ps)
                    for mt, jc, ps, ev in banks:
                        mm_aug(mt, jc, ps)
                    for mt, jc, ps, ev in banks:
                        evac(mt, jc, ps, ev, direct=False)
                else:
                    # bank-major: each bank runs its full accumulation chain
                    # and is evacuated immediately, so PSUM banks recycle
                    # continuously and pass boundaries never stall.
                    for mt, jc, ps, ev in banks:
                        for t in range(KT2):
                            mm(pc, t, mt, jc, ps)
                        mm_aug(mt, jc, ps)
                        evac(mt, jc, ps, ev, direct=(pc == last))

            osb = const.tile([P, 2 * MT], F32)
            for mt in range(MT):
                nc.vector.tensor_reduce(
                    osb[:, mt : mt + 1], minb[mt][:, :], AX, OP.min
                )
                nc.vector.tensor_reduce(
                    osb[:, MT + mt : MT + mt + 1], maxb[mt][:, :], AX, OP.max
                )
            nc.sync.dma_start(outd[:, :], osb[:, :])

    nc.finalize()
    return nc


@functools.lru_cache(maxsize=1)
def _get_program() -> bass.Bass:
    return _build()


def _make_in_maps(m1, m2, targets):
    ids = np.arange(NIDS)
    tgt = np.asarray(targets).astype(np.int64).reshape(N)

    # k-major fp8 operand layout: tile[p, s, x] = X[x, s*128 + p],
    # then split column-wise into contiguous pass-major blocks
    m2dr = np.ascontiguousarray(
        m2.astype(NP_F8).reshape(N, KT, P).transpose(2, 1, 0)
    )
    m2blocks = {
        f"m2p{pc}": np.ascontiguousarray(
            m2dr[:, :, PASS_O[pc] * CHUNK : (PASS_O[pc] + w) * CHUNK]
        )
        for pc, w in enumerate(PASS_W)
    }

    # right aug features (shared): one-hot ids + sq2 hi/lo (bf16 pair)
    sq2h = -0.5 * np.sum(m2.astype(np.float64) * m2.astype(np.float64), axis=1)
    shi = sq2h.astype(np.float32).astype(NP_BF16)
    slo = (sq2h - shi.astype(np.float64)).astype(np.float32).astype(NP_BF16)
    raug = np.zeros((KAUG, N), dtype=NP_BF16)
    raug[:NIDS] = (tgt[None, :] == ids[:, None]).astype(NP_BF16)
    raug[NIDS] = shi
    raug[NIDS + 1] = slo
    raug = np.ascontiguousarray(raug)

    maps = []
    for c in range(NCORES):
        m1c = m1[c * SH : (c + 1) * SH]
        m1dr = np.ascontiguousarray(
            m1c.astype(NP_F8).reshape(SH, KT, P).transpose(2, 1, 0)
        )
        tgtc = tgt[c * SH : (c + 1) * SH]
        laug = np.zeros((KAUG, SH), dtype=NP_BF16)
        laug[:NIDS] = (-BIG / 2.0) * (tgtc[None, :] == ids[:, None]).astype(
            np.float32
        )
        laug[NIDS] = 1.0
        laug[NIDS + 1] = 1.0
        maps.append(
            {
                "m1dr": m1dr,
                "laug": np.ascontiguousarray(laug),
                "raug": raug,
                **m2blocks,
            }
        )
    return maps


def run(modal1_inputs, modal2_inputs, targets, margin, trace=False):
    m1 = np.ascontiguousarray(np.asarray(modal1_inputs, dtype=np.float32))
    m2 = np.ascontiguousarray(np.asarray(modal2_inputs, dtype=np.float32))
    nc = _get_program()
    res = run_bass_kernel_spmd(
        nc, _make_in_maps(m1, m2, targets), list(range(NCORES)), trace=trace
    )

    # host finale: undo the psum encoding, sqrt, hinge, mean
    pmin = np.empty(N, dtype=np.float64)
    pmax = np.empty(N, dtype=np.float64)
    for c, r in enumerate(res.results):
        o = np.asarray(r["out"], dtype=np.float64)  # [P, 2*MT]
        pmin[c * SH : (c + 1) * SH] = o[:, :MT].T.reshape(SH)
        pmax[c * SH : (c + 1) * SH] = o[:, MT:].T.reshape(SH)

    sq1 = np.sum(m1.astype(np.float64) * m1.astype(np.float64), axis=1)
    ap2 = np.maximum(sq1 - 2.0 * pmin - BIG, EPS)
    an2 = np.maximum(sq1 - 2.0 * pmax, EPS)
    ap = np.sqrt(ap2)
    an = np.sqrt(an2)
    loss = np.float32(np.mean(np.maximum(ap - an + float(margin), 0.0)))
    prec = np.float32(np.mean(an > ap))
    return (loss, prec), res


def kernel(modal1_inputs, modal2_inputs, targets, margin):
    (loss, prec), _ = run(modal1_inputs, modal2_inputs, targets, margin)
    return loss, prec
